# revision 37
# baseline (speedup 1.0000x reference)
"""Trainium2 Bass kernel for a decoder-only transformer forward pass.

Reference semantics (see problem): the layer loop never feeds its output
back, so only the LAST layer's block matters:
    h   = E[x] + pos_encoding                         [B, S, D]
    Q/K/V = h @ Wq/k/v + b                            (last layer)
    att = softmax(causal(QK^T/sqrt(dk))) @ V
    h2  = att @ Wo + bo
    dec = tanh(h2 @ W1 + b1)                          [B, S, M]
    out = dec @ Wout + bout                           [B, S, OMEGA]

Sharding: 8 cores; core c handles batch b=c//4 and k=c%4, owning four
128-row q-tiles {k, 4+k, 8+k, 12+k} (of 16 tiles per batch), one from
each quarter of the sequence.  Every core computes K/V for its whole
batch (redundant x4) and its own 512-row slice of everything else,
including the dominant output-head matmul.  No collectives needed.

Causal slot-quantization: q-slot s (tile 4s+k) only needs keys
< 512*(s+1), so score/attV matmuls for key-chunk kk cover only the
column suffix [128*(kk//4), 512) — 62.5% of the full rectangle — while
every core runs the IDENTICAL program (only mask / gather data differ,
as SPMD requires).  The mask multiply is needed only on slot kk//4's own
128 columns (other active slots are strictly later rows, fully allowed).

bv is folded host-side into bo_eff = bo + bv @ Wo (exact: softmax rows
sum to 1), and bout arrives pre-broadcast as a [128, V] HBM tensor, so
no PE bias matmuls remain except the per-head reciprocal broadcast.

Device dataflow keeps activations TRANSPOSED (feature dim on partitions)
until the output head, so no on-device transposes are ever needed:
  hT -> QT/KT (lhsT = W in native layout), V (lhsT = hT)
  ST[k,q] = K.Q per head, P = exp(ST/sqrt(dk)) * causal_mask
  attT_unnorm[d',q] via lhsT=V; an appended ones-column in V yields the
  softmax denominator for free; normalize via reciprocal + ones-matmul
  partition-broadcast.
  h2T = Wo-matmul, decT = tanh(W1-matmul), logits = decT^T-matmul (the
  head flips back to q-on-partitions so the output DMA is contiguous).

All matmul operands are bf16 (fp32 PSUM accumulation).
"""

import sys

sys.path.insert(0, "/opt/trn_rl_repo")

import numpy as np
import ml_dtypes

import concourse.bass as bass  # noqa: F401
import concourse.mybir as mybir
import concourse.tile as tile
from concourse import bacc
from concourse.bass_utils import run_bass_kernel_spmd

P = 128
BF16 = mybir.dt.bfloat16
F32 = mybir.dt.float32


class Cfg:
    def __init__(self, S=2048, D=1024, DM=4096, V=16000, SQ=512, DK=64):
        self.S, self.D, self.DM, self.V, self.SQ, self.DK = S, D, DM, V, SQ, DK
        self.H = D // DK          # heads
        self.DC = D // P          # d chunks
        self.MC = DM // P         # ffn chunks
        self.KC = S // P          # key chunks
        self.QC = SQ // P         # q chunks (also causal slots per core)
        self.VW = 500             # head vocab block width (V = 32 * 500)
        self.VB = V // self.VW    # head vocab blocks


FULL = Cfg()


def _blocks(total, w=512):
    out = []
    v0 = 0
    while v0 < total:
        out.append((v0, min(w, total - v0)))
        v0 += w
    return out


def build_nc(cfg=FULL, debug=False, phases=4):
    c = cfg
    nc = bacc.Bacc("TRN2", target_bir_lowering=False, debug=debug)

    # weights grouped by OUTPUT chunk so they can be streamed per chunk:
    # wq[ki, out_ch, kc, kj] = Wq[kc*P+ki, out_ch*P+kj]
    hT = nc.dram_tensor("hT", [P, c.DC, c.S], BF16, kind="ExternalInput")
    hTq = nc.dram_tensor("hTq", [P, c.DC, c.SQ], BF16, kind="ExternalInput")
    wq = nc.dram_tensor("wq", [P, c.DC, c.DC, P], BF16, kind="ExternalInput")
    wk = nc.dram_tensor("wk", [P, c.DC, c.DC, P], BF16, kind="ExternalInput")
    wv = nc.dram_tensor("wv", [P, c.DC, c.D], BF16, kind="ExternalInput")
    wo = nc.dram_tensor("wo", [P, c.DC, c.D], BF16, kind="ExternalInput")
    w1 = nc.dram_tensor("w1", [P, c.MC, c.DC, P], BF16, kind="ExternalInput")
    # wout pre-tiled for long DMA lines: [ki, vb, mch, j] flattened
    wout = nc.dram_tensor("wout", [P, c.VB * c.MC * c.VW], BF16,
                          kind="ExternalInput")
    mask = nc.dram_tensor("mask", [P, c.KC, P], BF16, kind="ExternalInput")
    bqp = nc.dram_tensor("bqp", [P, c.DC], F32, kind="ExternalInput")
    bkp = nc.dram_tensor("bkp", [P, c.DC], F32, kind="ExternalInput")
    bop = nc.dram_tensor("bop", [P, c.DC], F32, kind="ExternalInput")
    b1p = nc.dram_tensor("b1p", [P, c.MC], F32, kind="ExternalInput")
    boutb = nc.dram_tensor("boutb", [P, c.V], BF16, kind="ExternalInput")
    # bf16 output halves the HBM write traffic; host upcasts to f32
    out = nc.dram_tensor("out", [c.SQ, c.V], BF16, kind="ExternalOutput")

    scale = 1.0 / np.sqrt(np.float32(c.DK))

    with tile.TileContext(nc) as tc:
        with (
            tc.tile_pool(name="const", bufs=1) as const,
            tc.tile_pool(name="persistB", bufs=1) as persistB,
        ):
            ones = const.tile([1, P], BF16, tag="ones")
            nc.any.memset(ones[:], 1.0)
            bq_sb = const.tile([P, c.DC], F32, tag="bq")
            bk_sb = const.tile([P, c.DC], F32, tag="bk")
            bo_sb = const.tile([P, c.DC], F32, tag="bo")
            b1_sb = const.tile([P, c.MC], F32, tag="b1")
            nc.sync.dma_start(bq_sb[:], bqp[:])
            nc.sync.dma_start(bk_sb[:], bkp[:])
            nc.sync.dma_start(bo_sb[:], bop[:])
            nc.sync.dma_start(b1_sb[:], b1p[:])

            attn_sb = persistB.tile([P, c.DC, c.SQ], BF16, tag="attn")
            dec_sb = persistB.tile([P, c.MC, c.SQ], BF16, tag="dec")

            with tc.tile_pool(name="persistA", bufs=1) as persistA:
                qt_sb = persistA.tile([P, c.DC, c.SQ], BF16, tag="qt")
                kt_sb = persistA.tile([P, c.DC, c.S], BF16, tag="kt")
                # V with an appended ones-column per head: [P, H, DK+1]
                v_sb = [
                    persistA.tile([P, c.H, c.DK + 1], BF16, tag=f"v{kk}",
                                  name=f"v{kk}")
                    for kk in range(c.KC)
                ]

                # ---- phases 1+2 share one scope: projections + attention
                # V first, then KT/QT interleaved per chunk, so head h can
                # start (needs chunk h//2 only) while later projections
                # still run — the ACT-bound softmax stretch overlaps the
                # PE-bound projection tail.
                with (
                    tc.tile_pool(name="wstream", bufs=3) as wstream,
                    tc.tile_pool(name="wvpool", bufs=1) as wvpool,
                    tc.tile_pool(name="hpool", bufs=1) as hpool,
                    tc.tile_pool(name="maskp", bufs=1) as maskp,
                    tc.tile_pool(name="ppool", bufs=3) as ppool,
                    tc.tile_pool(name="npool", bufs=1) as npool,
                    tc.tile_pool(name="ps1", bufs=2, space="PSUM") as ps1,
                    tc.tile_pool(name="st_ps", bufs=2, space="PSUM") as st_ps,
                    tc.tile_pool(name="at_ps", bufs=2, space="PSUM") as at_ps,
                ):
                    wv_sb = [
                        wvpool.tile([P, c.D], BF16, tag=f"wv{kc}",
                                    name=f"wv{kc}")
                        for kc in range(c.DC)
                    ]
                    hT_sb = [
                        hpool.tile([P, c.S], BF16, tag=f"hT{kc}",
                                   name=f"hT{kc}")
                        for kc in range(c.DC)
                    ]
                    hTq_sb = hpool.tile([P, c.DC, c.SQ], BF16, tag="hTq")
                    wq_sb = hpool.tile([P, c.DC, c.DC, P], BF16, tag="wq")
                    mask_sb = maskp.tile([P, c.KC, P], BF16, tag="mask")
                    # DMA issue order is execution order: hTq+wq (needed by
                    # QT, ~3MB) go first so QT starts within a few us and
                    # covers the hT/wv load window; per-chunk hT/wv tiles
                    # let the V/KT chains start as soon as chunk 0 lands.
                    nc.sync.dma_start(hTq_sb[:], hTq[:])
                    nc.sync.dma_start(wq_sb[:, 0], wq[:, 0])
                    nc.sync.dma_start(wq_sb[:, 1:], wq[:, 1:])
                    for ch in range(c.DC):
                        nc.sync.dma_start(hT_sb[ch][:], hT[:, ch, :])
                        nc.sync.dma_start(wv_sb[ch][:], wv[:, ch, :])
                    nc.sync.dma_start(mask_sb[:], mask[:])

                    # QT (+bq) first — fills the PE while hT/wv stream in
                    for ch in range(c.DC):
                        ps = ps1.tile([P, 512], F32, tag="ps1t")
                        for kc in range(c.DC):
                            nc.tensor.matmul(
                                ps[:, : c.SQ],
                                wq_sb[:, ch, kc, :],
                                hTq_sb[:, kc, :],
                                start=(kc == 0),
                                stop=(kc == c.DC - 1),
                            )
                        nc.vector.tensor_scalar_add(
                            qt_sb[:, ch, :], ps[:, : c.SQ],
                            bq_sb[:, ch : ch + 1],
                        )

                    # V[k,d'] native; lhsT = hT key-chunk, rhs = Wv
                    # (bv is folded into bo_eff host-side)
                    for kk in range(c.KC):
                        nc.any.memset(v_sb[kk][:, :, c.DK : c.DK + 1], 1.0)
                        for (d0, W) in _blocks(c.D):
                            ps = ps1.tile([P, 512], F32, tag="ps1t")
                            for kc in range(c.DC):
                                nc.tensor.matmul(
                                    ps[:, :W],
                                    hT_sb[kc][:, kk * P : (kk + 1) * P],
                                    wv_sb[kc][:, d0 : d0 + W],
                                    start=(kc == 0),
                                    stop=(kc == c.DC - 1),
                                )
                            h0 = d0 // c.DK
                            h1 = (d0 + W) // c.DK
                            nc.vector.tensor_copy(
                                v_sb[kk][:, h0:h1, 0 : c.DK],
                                ps[:, :W].rearrange("p (h d) -> p h d",
                                                    d=c.DK),
                            )

                    # KT (all S keys, +bk) per chunk so heads unblock
                    # progressively
                    for ch in range(c.DC):
                        wt = wstream.tile([P, c.DC, P], BF16, tag="wt")
                        nc.sync.dma_start(wt[:], wk[:, ch])
                        for (n0, W) in _blocks(c.S):
                            ps = ps1.tile([P, 512], F32, tag="ps1t")
                            for kc in range(c.DC):
                                nc.tensor.matmul(
                                    ps[:, :W],
                                    wt[:, kc, :],
                                    hT_sb[kc][:, n0 : n0 + W],
                                    start=(kc == 0),
                                    stop=(kc == c.DC - 1),
                                )
                            nc.vector.tensor_scalar_add(
                                kt_sb[:, ch, n0 : n0 + W],
                                ps[:, :W],
                                bk_sb[:, ch : ch + 1],
                            )

                    # ---------------- attention ----------------
                    # causal slot-quantization: key-chunk kk only feeds
                    # q-slots >= kk//4, i.e. columns [128*(kk//4), 512);
                    # the mask multiply is only needed on slot kk//4's own
                    # 128 columns (later slots are fully allowed).
                    def _finish_head(pend):
                        p_at, p_rbf, p_hp, p_ch = pend
                        nc.tensor.matmul(
                            p_at[c.DK : 2 * c.DK, :], ones[0:1, 0 : c.DK],
                            p_rbf[:],
                            start=True, stop=True,
                        )
                        # hw allows only one PSUM operand per DVE op, so
                        # the broadcast bounces through SBUF
                        rb_sb = npool.tile([c.DK, c.SQ], F32, tag="rb_sb")
                        nc.vector.tensor_copy(
                            rb_sb[:], p_at[c.DK : 2 * c.DK, :]
                        )
                        nc.vector.tensor_tensor(
                            attn_sb[p_hp : p_hp + c.DK, p_ch, :],
                            p_at[0 : c.DK, :],
                            rb_sb[:],
                            mybir.AluOpType.mult,
                        )

                    pending = None
                    # kk pairs share one 2-bank PSUM tile so a single exp
                    # covers both (halves the ACT per-op overhead, which
                    # gates the attention span)
                    for h in range(c.H if phases >= 2 else 0):
                        hp = 64 * (h % 2)
                        ch = h // 2
                        at = at_ps.tile([P, c.SQ], F32, tag="at")
                        p_tiles = []
                        for kk2 in range(c.KC // 2):
                            c0 = P * ((2 * kk2) // c.QC)
                            W = c.SQ - c0
                            st = st_ps.tile([P, 2, c.SQ], F32, tag="st")
                            for j in (0, 1):
                                kk = 2 * kk2 + j
                                nc.tensor.matmul(
                                    st[:, j, :W],
                                    kt_sb[hp : hp + c.DK, ch,
                                          kk * P : (kk + 1) * P],
                                    qt_sb[hp : hp + c.DK, ch, c0:],
                                    start=True,
                                    stop=True,
                                )
                            p = ppool.tile([P, 2, c.SQ], BF16, tag="p")
                            nc.scalar.activation(
                                p[:, :, c0:], st[:, :, :W],
                                mybir.ActivationFunctionType.Exp,
                                scale=float(scale),
                            )
                            for j in (0, 1):
                                kk = 2 * kk2 + j
                                nc.vector.tensor_tensor(
                                    p[:, j, c0 : c0 + P],
                                    p[:, j, c0 : c0 + P],
                                    mask_sb[:, kk, :],
                                    mybir.AluOpType.mult,
                                )
                            p_tiles.append(p)
                        # single accumulation group: kk=0 (c0=0) zeroes the
                        # whole tile via start; later kk accumulate into
                        # their active column suffix only
                        for kk in range(c.KC):
                            c0 = P * (kk // c.QC)
                            nc.tensor.matmul(
                                at[0 : c.DK + 1, c0:],
                                v_sb[kk][:, h, :],
                                p_tiles[kk // 2][:, kk % 2, c0:],
                                start=(kk == 0),
                                stop=(kk == c.KC - 1),
                            )
                        # normalize by the ones-column sum (row DK of at).
                        # The reciprocal broadcast lands in at's dead rows
                        # [64:128] (row 64's denominator is consumed by the
                        # reciprocal first), so no extra PSUM bank is needed.
                        # The DVE half (recip+downcast) is emitted now; the
                        # PE rb-matmul + final mult are DEFERRED until after
                        # the next head's score/attV matmuls so the in-order
                        # PE never stalls waiting on the DVE chain.
                        r32 = npool.tile([1, c.SQ], F32, tag="r32")
                        nc.vector.reciprocal(r32[:], at[c.DK : c.DK + 1, :])
                        rbf = npool.tile([1, c.SQ], BF16, tag="rbf")
                        nc.vector.tensor_copy(rbf[:], r32[:])
                        if pending is not None:
                            _finish_head(pending)
                        pending = (at, rbf, hp, ch)
                    if pending is not None and phases >= 2:
                        _finish_head(pending)

            # ------- phases 3+4 share one scope: Wo + FFN + output head ----
            # (lets the first Wout slab DMAs and head matmuls overlap the
            # FFN tail instead of waiting for a pool boundary)
            with (
                tc.tile_pool(name="wopool", bufs=1) as wopool,
                tc.tile_pool(name="w1stream", bufs=3) as w1stream,
                tc.tile_pool(name="h2pool", bufs=1) as h2pool,
                tc.tile_pool(name="ps3", bufs=3, space="PSUM") as ps3,
                tc.tile_pool(name="wout_p", bufs=4) as wout_p,
                tc.tile_pool(name="bout_p", bufs=4) as bout_p,
                tc.tile_pool(name="out_p", bufs=4) as out_p,
                tc.tile_pool(name="ps4", bufs=4, space="PSUM") as ps4,
            ):
                wo_sb = wopool.tile([P, c.DC, c.D], BF16, tag="wo")
                for ch in range(c.DC):
                    nc.sync.dma_start(wo_sb[:, ch, :], wo[:, ch, :])
                h2_sb = h2pool.tile([P, c.DC, c.SQ], BF16, tag="h2")
                for ch in range(c.DC if phases >= 3 else 0):
                    ps = ps3.tile([P, 512], F32, tag="ps3t")
                    for kc in range(c.DC):
                        nc.tensor.matmul(
                            ps[:, : c.SQ],
                            wo_sb[:, kc, ch * P : (ch + 1) * P],
                            attn_sb[:, kc, :],
                            start=(kc == 0),
                            stop=(kc == c.DC - 1),
                        )
                    nc.vector.tensor_scalar_add(
                        h2_sb[:, ch, :], ps[:, : c.SQ], bo_sb[:, ch : ch + 1]
                    )
                for mch in range(c.MC if phases >= 3 else 0):
                    wt = w1stream.tile([P, c.DC, P], BF16, tag="w1t")
                    nc.sync.dma_start(wt[:], w1[:, mch])
                    ps = ps3.tile([P, 512], F32, tag="ps3t")
                    for kc in range(c.DC):
                        nc.tensor.matmul(
                            ps[:, : c.SQ],
                            wt[:, kc, :],
                            h2_sb[:, kc, :],
                            start=(kc == 0),
                            stop=(kc == c.DC - 1),
                        )
                    nc.scalar.activation(
                        dec_sb[:, mch, :], ps[:, : c.SQ],
                        mybir.ActivationFunctionType.Tanh,
                        bias=b1_sb[:, mch : mch + 1],
                        scale=1.0,
                    )

                # ---------------- phase 4: output head ----------------
                # vocab blocks processed in groups of G: the G matmuls per
                # (qc, mch) share the same stationary dec tile back-to-back,
                # cutting PE weight-reload traffic ~Gx (the sim doesn't
                # model Ldweights; real hardware pays for it)
                W = c.VW
                G = 2
                vgroups = [list(range(g, min(g + G, c.VB)))
                           for g in range(0, c.VB, G)]
                for vg in (vgroups if phases >= 4 else []):
                    wts, bbs = [], []
                    for vb in vg:
                        off = vb * c.MC * W
                        wt = wout_p.tile([P, c.MC, W], BF16, tag="wt4")
                        step = max(1, c.MC // 4)
                        for m0 in range(0, c.MC, step):
                            m1 = min(c.MC, m0 + step)
                            nc.sync.dma_start(
                                wt[:, m0:m1, :],
                                wout[:, off + m0 * W : off + m1 * W],
                            )
                        bb_sb = bout_p.tile([P, W], BF16, tag="bb_sb")
                        nc.sync.dma_start(bb_sb[:], boutb[:, vb * W : vb * W + W])
                        wts.append(wt)
                        bbs.append(bb_sb)
                    for qc in range(c.QC):
                        pss = [ps4.tile([P, 512], F32, tag="ps4t",
                                        name=f"ps4_{gi}")
                               for gi in range(len(vg))]
                        for mch in range(c.MC):
                            for gi in range(len(vg)):
                                nc.tensor.matmul(
                                    pss[gi][:, :W],
                                    dec_sb[:, mch, qc * P : (qc + 1) * P],
                                    wts[gi][:, mch, :],
                                    start=(mch == 0),
                                    stop=(mch == c.MC - 1),
                                )
                        for gi, vb in enumerate(vg):
                            ot = out_p.tile([P, 512], BF16, tag="ot")
                            nc.vector.tensor_tensor(
                                ot[:, :W], pss[gi][:, :W], bbs[gi][:],
                                mybir.AluOpType.add,
                            )
                            nc.sync.dma_start(
                                out[qc * P : (qc + 1) * P,
                                    vb * W : vb * W + W],
                                ot[:, :W],
                            )

    nc.compile()
    return nc


# ---------------------------------------------------------------------------
# host side
# ---------------------------------------------------------------------------

def _pos_encoding(seq_len, d):
    pos = np.arange(seq_len, dtype=np.float32)[:, None]
    div = np.exp(
        np.arange(0, d, 2, dtype=np.float32) * (-np.log(10000.0) / d)
    )
    pe = np.zeros((seq_len, d), dtype=np.float32)
    pe[:, 0::2] = np.sin(pos * div)
    pe[:, 1::2] = np.cos(pos * div)
    return pe


def _chunked(a, pdim_chunks):
    """[N, F] -> [128, N//128, F] with row n = ko*128+ki -> [ki, ko, f]."""
    n, f = a.shape
    return np.ascontiguousarray(
        a.reshape(pdim_chunks, P, f).transpose(1, 0, 2)
    )


def _grouped(a, kchunks, ochunks):
    """[K, O] -> [128, O//128, K//128, 128]: [ki, oc, kc, kj]."""
    k, o = a.shape
    return np.ascontiguousarray(
        a.reshape(kchunks, P, ochunks, P).transpose(1, 2, 0, 3)
    )


_NC_CACHE = {}


def _get_nc(cfg=FULL):
    key = (cfg.S, cfg.D, cfg.DM, cfg.V, cfg.SQ)
    if key not in _NC_CACHE:
        _NC_CACHE[key] = build_nc(cfg)
    return _NC_CACHE[key]


def make_in_maps(x, E, Wq, bq, Wk, bk, Wv, bv, Wo, bo, W1, b1, Wout, bout,
                 cfg=FULL, n_cores=8):
    c = cfg
    bf = ml_dtypes.bfloat16
    x = np.asarray(x)
    E = np.asarray(E, dtype=np.float32)
    B = x.shape[0]
    h = E[x] + _pos_encoding(x.shape[1], E.shape[1])[None]

    wq_a = _grouped(np.asarray(Wq[-1]).astype(bf), c.DC, c.DC)
    wk_a = _grouped(np.asarray(Wk[-1]).astype(bf), c.DC, c.DC)
    wv_a = _chunked(np.asarray(Wv[-1]).astype(bf), c.DC)
    wo_a = _chunked(np.asarray(Wo[-1]).astype(bf), c.DC)
    w1_a = _grouped(np.asarray(W1[-1]).astype(bf), c.DC, c.MC)
    # wout tiled [ki, vb, mch, j] then flattened to [128, VB*MC*VW]
    wout_a = np.ascontiguousarray(
        np.asarray(Wout).astype(bf)
        .reshape(c.MC, P, c.VB, c.VW)
        .transpose(1, 2, 0, 3)
        .reshape(P, c.VB * c.MC * c.VW)
    )
    f32 = np.float32
    # fold bv into bo: softmax rows sum to 1, so att = att_nobias + bv
    # and (att_nobias + bv) @ Wo + bo = att_nobias @ Wo + (bv @ Wo + bo)
    bo_eff = (np.asarray(bo[-1], f32)
              + np.asarray(bv[-1], f32) @ np.asarray(Wo[-1], f32))
    bq_a = np.ascontiguousarray(np.asarray(bq[-1]).reshape(c.DC, P).T).astype(f32)
    bk_a = np.ascontiguousarray(np.asarray(bk[-1]).reshape(c.DC, P).T).astype(f32)
    bo_a = np.ascontiguousarray(bo_eff.reshape(c.DC, P).T).astype(f32)
    b1_a = np.ascontiguousarray(np.asarray(b1[-1]).reshape(c.MC, P).T).astype(f32)
    bout_a = np.ascontiguousarray(
        np.broadcast_to(np.asarray(bout).astype(bf)[None, :], (P, c.V))
    )

    hT_b = [_chunked(np.ascontiguousarray(h[b].T).astype(bf), c.DC)
            for b in range(B)]

    groups_per_batch = n_cores // B
    ki = np.arange(P)
    in_maps = []
    for core in range(n_cores):
        b = core // groups_per_batch
        k = core % groups_per_batch
        # q-tiles {k, 4+k, 8+k, 12+k}; slot s holds tile QC*s + k
        tiles = [c.QC * s + k for s in range(c.QC)]
        qidx = np.concatenate(
            [np.arange(P * t, P * t + P) for t in tiles]
        )
        # mask[ki, kk, qi]: 1 iff q_row >= key, where for key-chunk kk the
        # masked columns belong to slot kk//QC's tile t = QC*(kk//QC) + k
        m = np.empty((P, c.KC, P), np.float32)
        for kk in range(c.KC):
            t = c.QC * (kk // c.QC) + k
            m[:, kk, :] = (
                (P * t + ki)[None, :] >= (P * kk + ki)[:, None]
            )
        in_maps.append({
            "hT": hT_b[b],
            "hTq": np.ascontiguousarray(hT_b[b][:, :, qidx]),
            "wq": wq_a, "wk": wk_a, "wv": wv_a, "wo": wo_a, "w1": w1_a,
            "wout": wout_a,
            "mask": np.ascontiguousarray(m.astype(bf)),
            "bqp": bq_a, "bkp": bk_a, "bop": bo_a, "b1p": b1_a,
            "boutb": bout_a,
        })
    return in_maps


def kernel(x, E, Wq, bq, Wk, bk, Wv, bv, Wo, bo, W1, b1, Wout, bout,
           num_heads=16, **kw):
    c = FULL
    assert int(num_heads) == c.H
    x = np.asarray(x)
    nc = _get_nc(c)
    in_maps = make_in_maps(x, E, Wq, bq, Wk, bk, Wv, bv, Wo, bo, W1, b1,
                           Wout, bout, cfg=c)
    try:
        res = run_bass_kernel_spmd(nc, in_maps, core_ids=list(range(8)))
    except Exception:
        # a previous session may have left a NeuronCore wedged
        # (NRT_EXEC_UNIT_UNRECOVERABLE); give the runtime time to reset
        # and retry once
        import time as _time
        _time.sleep(60)
        res = run_bass_kernel_spmd(nc, in_maps, core_ids=list(range(8)))
    B = x.shape[0]
    S = x.shape[1]
    out = np.empty((B, S, c.V), np.float32)
    groups_per_batch = 8 // B
    for core in range(8):
        b = core // groups_per_batch
        k = core % groups_per_batch
        r = np.asarray(res.results[core]["out"], dtype=np.float32)
        for s in range(c.QC):
            t = c.QC * s + k
            out[b, P * t : P * t + P] = r[P * s : P * s + P]
    return out

